# revision 18
# baseline (speedup 1.0000x reference)
"""GNN message-passing kernel for TRN2 (8 NeuronCores, SPMD).

Math (see reference):
  h = relu(x @ W_in + b_in);  h = LayerNorm(h) * ln_g + ln_b
  deg/dinv from edge_src;  hn = h / (||h|| + 1e-4)
  for 3 layers:
     ang_i = sum_{e: src=i} dinv_src*dinv_dst*<hn_src, hn_dst>   (clip never binds)
     rotate hn[:,0:2] by ang (Givens)
  z = relu(h @ cW1 + cb1); bn-affine; logits = z @ cW2 + cb2; log_softmax

Algebraic restructuring (same as v1):
  - Givens rotation preserves ||h||; only hn[:,0:2] changes across layers.
  - ang_i = <hn_i, M_i>, M_i = sum_e w_e * hn_dst  (w_e = dinv_src*dinv_dst)
  - T_i (tail part, dims 2:512 + layer-1 head) computed once; layers 2,3 only
    need P_i,Q_i = sum_e w_e * (a,b)_dst with fresh head values.

v2 changes (all informed by the TimelineSim cost model):
  - Phase 0 (dense+LN+normalize) sharded: each core computes only its own
    6272 nodes, then one AllGather of the fp8 feature table Y (25.7 MB out).
    v1 replicated this work 8x on every core.
  - Y stored fp8e4 (512 B rows): halves phase-3 gather bytes; PE matmul cost
    keys on the moving operand, so rhs fp8 runs at bf16 speed.
  - Diagonal slot assignment: slot (p, b) holds an edge with src_local == p
    for the first D0/D1 blocks per class; only ~6 overflow blocks per group
    need a general selection matrix.  The per-group selection build drops
    from 2 full [P,BT,P] DVE ops x 3 phases to ~1 equivalent op once, and
    layers 2/3 reduce the diagonal part on DVE directly (no PE, no selw).
  - Layers 2/3 gather ONE pair-packed table (row = (a,b) of nodes 2k,2k+1,
    256 B min row) instead of two 256 B-row tables: half the descriptors.
  - bn affine folded into cW2/cb2 on host; classifier matmuls via f32r
    (1 cyc/row at free>=256 vs 4 for plain f32).
  - hn kept resident in SBUF (bf16) -- no DRAM round trip.

Distribution: nodes sharded contiguously in GLOBAL order (6272/core, padded
to 50176).  Edges partitioned by src core.  Gather tables are indexed with
global node ids (int16 via lo/hi split for Y, via pair-row ids for ab).
"""

import math
import numpy as np
import ml_dtypes

import sys as _sys
for _p in ("/opt/trn_rl_repo", "/root/.axon_site/_ro/trn_rl_repo"):
    if _p not in _sys.path:
        _sys.path.insert(0, _p)
import concourse.bacc as bacc
import concourse.tile as tile
import concourse.bass as bass
import concourse.mybir as mybir
from concourse.masks import make_identity

dt = mybir.dt
P = 128
D = 512
DOUT = 40
LN_EPS = 1e-5
BN_EPS = 1e-5
NRM_EPS = 1e-4
DIAG = 15          # diagonal blocks per class
UVW = 64           # uvpair row width in f32 (256 B)


class Cfg:
    def __init__(self, n_cores, gpc, ov, flags, vb=8):
        self.NC = n_cores
        self.GPC = gpc                   # groups (of 128 nodes) per core
        self.NPC = gpc * P               # nodes per core
        self.NPAD = n_cores * self.NPC
        self.HALF = self.NPAD // 2
        self.OV = ov                     # dict ycls -> overflow blocks
        self.B = {y: DIAG + ov[y] for y in (0, 1)}
        self.BT = self.B[0] + self.B[1]
        self.VB = vb                     # phase-0 block batch
        self.flags = flags


# ---------------------------------------------------------------- host prep

def _cumcount(keys):
    """Per-element rank within its key group (keys need not be sorted)."""
    order = np.argsort(keys, kind="stable")
    ks = keys[order]
    starts = np.concatenate([[0], np.flatnonzero(ks[1:] != ks[:-1]) + 1])
    grp = np.zeros(len(ks), np.int64)
    grp[starts] = 1
    gid = np.cumsum(grp) - 1
    rank_sorted = np.arange(len(ks)) - starts[gid]
    rank = np.empty(len(ks), np.int64)
    rank[order] = rank_sorted
    return rank


def host_prep(x, edge_src, edge_dst, n_cores=8, gpc=None):
    """Build per-core inputs + slot/index arrays. Returns (cfg, percore)."""
    N = x.shape[0]
    edge_src = edge_src.astype(np.int64)
    edge_dst = edge_dst.astype(np.int64)
    if gpc is None:
        gpc = (N + n_cores * P - 1) // (n_cores * P)
    NPC = gpc * P
    NPAD = n_cores * NPC
    HALF = NPAD // 2
    assert HALF % P == 0

    deg = np.bincount(edge_src, minlength=N).astype(np.float64)
    dinv = np.where(deg > 0, deg ** -0.5, 0.0).astype(np.float32)
    w_all = dinv[edge_src] * dinv[edge_dst]          # per-edge weight

    src_core = edge_src // NPC
    # ---- global overflow block count (must be uniform across cores) ----
    ycls_all = (edge_dst >= HALF).astype(np.int64)
    p_all = edge_src % P
    gg_all = edge_src // P                           # global group id
    key_all = (gg_all * 2 + ycls_all) * P + p_all
    rank_all = _cumcount(key_all)
    ov = {}
    for y in (0, 1):
        m = (ycls_all == y) & (rank_all >= DIAG)
        cnt = np.bincount(gg_all[m], minlength=n_cores * gpc)
        ov[y] = max(1, int(np.ceil(cnt.max() / P)))
    B = {y: DIAG + ov[y] for y in (0, 1)}
    BT = B[0] + B[1]
    ybase = {0: 0, 1: B[0]}                          # block base per class
    su_y = {y: B[y] * P // 16 for y in (0, 1)}

    def wrap16(a2):      # [gpc, tot] int16 -> [gpc, 128, tot/16]
        w3 = a2.reshape(gpc, -1, 16).transpose(0, 2, 1)
        return np.ascontiguousarray(np.tile(w3, (1, 8, 1)))

    xpadT = np.zeros((D, NPAD), np.float32)
    xpadT[:, :N] = x.T

    percore = []
    for r in range(n_cores):
        m = src_core == r
        es = edge_src[m]
        ed = edge_dst[m]
        ww = w_all[m].astype(np.float32)
        g = (es - r * NPC) // P
        p = es % P
        ycls = (ed >= HALF).astype(np.int64)
        rank = _cumcount((g * 2 + ycls) * P + p)

        # slot (g, sp, b) per edge
        sp = np.empty(len(es), np.int64)
        b = np.empty(len(es), np.int64)
        dm = rank < DIAG
        sp[dm] = p[dm]
        base_v = np.where(ycls == 0, ybase[0], ybase[1])
        b[dm] = base_v[dm] + rank[dm]
        # overflow: sequential slots per (g, y)
        om = ~dm
        ovr = _cumcount((g[om] * 2 + ycls[om]))
        sp[om] = ovr % P
        b[om] = base_v[om] + DIAG + ovr // P
        assert (b[om] < base_v[om] + B[0] * (ycls[om] == 0) + B[1] * (ycls[om] == 1)).all()

        flat = (g * P + sp) * BT + b                 # [gpc, P, BT] linear
        omg = np.zeros(gpc * P * BT, np.float32)
        podd = np.zeros(gpc * P * BT, np.float32)
        srcl = np.zeros(gpc * P * BT, np.float32)
        uidxf = np.zeros(gpc * P * BT, np.int16)
        omg[flat] = ww
        podd[flat] = (ed & 1).astype(np.float32)
        srcl[flat] = p.astype(np.float32)
        uidxf[flat] = (ed >> 1).astype(np.int16)
        omg = omg.reshape(gpc, P, BT)
        podd = podd.reshape(gpc, P, BT)
        # srcl only needed for overflow blocks
        srcl_ov = np.concatenate(
            [srcl.reshape(gpc, P, BT)[:, :, DIAG:B[0]],
             srcl.reshape(gpc, P, BT)[:, :, B[0] + DIAG:BT]], axis=2)
        srcl_ov = np.ascontiguousarray(srcl_ov)

        # y gather indices: linear i = b_local*128 + sp, per class
        yidx = {}
        for y in (0, 1):
            arr = np.zeros(gpc * B[y] * P, np.int16)
            my = ycls == y
            bl = b[my] - ybase[y]
            lin = g[my] * (B[y] * P) + bl * P + sp[my]
            arr[lin] = (ed[my] - y * HALF).astype(np.int16)
            yidx[y] = wrap16(arr.reshape(gpc, B[y] * P))
        # uv gather indices over all BT blocks, linear i = b*128 + sp
        uarr = np.zeros(gpc * BT * P, np.int16)
        lin = g * (BT * P) + b * P + sp
        uarr[lin] = (ed >> 1).astype(np.int16)
        uidx = wrap16(uarr.reshape(gpc, BT * P))

        percore.append(dict(
            xT=np.ascontiguousarray(xpadT[:, r * NPC:(r + 1) * NPC]),
            omg=omg, podd=podd, srcl_ov=srcl_ov,
            yidx0=yidx[0], yidx1=yidx[1], uidx=uidx))

    cfg = Cfg(n_cores, gpc, ov, {})
    return cfg, percore


# ---------------------------------------------------------------- device build

def build_nc(cfg, skip_cc=False):
    NC, GPC, NPC, NPAD, HALF = cfg.NC, cfg.GPC, cfg.NPC, cfg.NPAD, cfg.HALF
    B, BT, OV, VB = cfg.B, cfg.BT, cfg.OV, cfg.VB
    OVT = OV[0] + OV[1]
    FL = cfg.flags
    NPAIR = NPAD // 2

    f32, f32r, bf16, fp8, i16 = (dt.float32, dt.float32r, dt.bfloat16,
                                 dt.float8e4, dt.int16)
    AF = mybir.ActivationFunctionType
    OP = mybir.AluOpType

    nc = bacc.Bacc("TRN2", target_bir_lowering=False, debug=False, num_devices=NC)

    # ---------------- I/O ----------------
    xT = nc.dram_tensor("xT", [D, NPC], f32, kind="ExternalInput").ap()
    W_in = nc.dram_tensor("W_in", [D, D], f32, kind="ExternalInput").ap()
    b_in = nc.dram_tensor("b_in", [1, D], f32, kind="ExternalInput").ap()
    ln_g = nc.dram_tensor("ln_g", [1, D], f32, kind="ExternalInput").ap()
    ln_b = nc.dram_tensor("ln_b", [1, D], f32, kind="ExternalInput").ap()
    cW1 = nc.dram_tensor("cW1", [D, D], bf16, kind="ExternalInput").ap()
    cb1 = nc.dram_tensor("cb1", [1, D], f32, kind="ExternalInput").ap()
    cW2 = nc.dram_tensor("cW2", [D, DOUT], bf16, kind="ExternalInput").ap()
    cb2 = nc.dram_tensor("cb2", [1, DOUT], f32, kind="ExternalInput").ap()
    omgT = nc.dram_tensor("omg", [GPC, P, BT], f32, kind="ExternalInput").ap()
    poddT = nc.dram_tensor("podd", [GPC, P, BT], f32, kind="ExternalInput").ap()
    srclovT = nc.dram_tensor("srcl_ov", [GPC, P, OVT], f32,
                             kind="ExternalInput").ap()
    yidxT = {}
    for y in (0, 1):
        s = B[y] * P // 16
        yidxT[y] = nc.dram_tensor(f"yidx{y}", [GPC, P, s], i16,
                                  kind="ExternalInput").ap()
    su = BT * P // 16
    uidxT = nc.dram_tensor("uidx", [GPC, P, su], i16, kind="ExternalInput").ap()
    out = nc.dram_tensor("out", [NPC, DOUT], f32, kind="ExternalOutput").ap()

    # ---------------- internal DRAM ----------------
    uv = nc.dram_tensor("uvtab", [NPAIR, UVW], f32, kind="Internal").ap()

    from contextlib import ExitStack
    with tile.TileContext(nc) as tc, ExitStack() as stack:
        pers = stack.enter_context(tc.tile_pool(name="pers", bufs=1))
        dram = stack.enter_context(tc.tile_pool(name="dram", bufs=1, space="DRAM"))

        # collective buffers
        ccy_in = dram.tile([NPC, D], fp8, tag="ccy_in")
        ccy_out = dram.tile([NC, NPC, D], fp8, tag="ccy_out")
        Yt = ccy_out[:].rearrange("r n e -> (r n) e")       # [NPAD, D]
        ccab_in = dram.tile([P, GPC * 2], f32, tag="ccab_in")
        ccab_out = dram.tile([NC, P, GPC * 2], f32, tag="ccab_out")

        # persistent tiles
        w_in_sb = pers.tile([P, 4, D], f32r)
        cw1_sb = pers.tile([P, 4, D], bf16)
        cw2_sb = pers.tile([P, 4, DOUT], bf16)
        iota_f = pers.tile([P, P], f32)
        ident = pers.tile([P, P], f32)
        ident_bf = pers.tile([P, P], bf16)
        halfpi = pers.tile([P, 1], f32)
        epsln = pers.tile([P, 1], f32)
        hn_all = pers.tile([P, GPC, D], bf16)      # 50 KB/part
        gml = bml = binm = cb1m = None
        if not FL.get("ln_trivial", False):
            gml = pers.tile([P, D], f32)     # ln gamma mat (general path)
            bml = pers.tile([P, D], f32)     # ln beta mat
        if not FL.get("bin_zero", True):
            binm = pers.tile([P, D], f32)
        if not FL.get("cb1_zero", True):
            cb1m = pers.tile([P, D], f32)
        cb2m = pers.tile([P, DOUT], f32)
        a_own = pers.tile([P, GPC], f32)
        b_own = pers.tile([P, GPC], f32)
        d_own = pers.tile([P, GPC], f32)
        T_own = pers.tile([P, GPC], f32)
        ang1 = pers.tile([P, GPC], f32)
        P_all = pers.tile([P, GPC], f32)
        Q_all = pers.tile([P, GPC], f32)
        c_t = pers.tile([P, GPC], f32)
        s_t = pers.tile([P, GPC], f32)
        r1 = pers.tile([P, GPC], f32)
        r2 = pers.tile([P, GPC], f32)
        r3 = pers.tile([P, GPC], f32)
        r4 = pers.tile([P, GPC], f32)
        angL = pers.tile([P, GPC], f32)
        uvp = pers.tile([P, GPC, 2], f32)
        omg_all = pers.tile([P, GPC, BT], f32)
        podd_all = pers.tile([P, GPC, BT], bf16)
        pinv_all = pers.tile([P, GPC, BT], bf16)
        srclov_all = pers.tile([P, GPC, OVT], f32)
        bnt1 = pers.tile([1, D], f32)

        # ---- one-time setup ----
        nc.sync.dma_start(out=w_in_sb[:], in_=W_in.rearrange("(k p) f -> p k f", k=4, p=P).bitcast(f32r))
        nc.sync.dma_start(out=cw1_sb[:], in_=cW1.rearrange("(k p) f -> p k f", k=4, p=P))
        nc.sync.dma_start(out=cw2_sb[:], in_=cW2.rearrange("(k p) f -> p k f", k=4, p=P))
        nc.sync.dma_start(out=omg_all[:], in_=omgT.rearrange("g p s -> p g s"))
        nc.sync.dma_start(out=srclov_all[:], in_=srclovT.rearrange("g p s -> p g s"))
        nc.gpsimd.dma_start(out=podd_all[:], in_=poddT.rearrange("g p s -> p g s"))
        nc.vector.tensor_scalar(out=pinv_all[:], in0=podd_all[:], scalar1=-1.0,
                                scalar2=1.0, op0=OP.mult, op1=OP.add)
        nc.gpsimd.memset(halfpi[:], math.pi / 2)
        nc.gpsimd.memset(epsln[:], LN_EPS)
        iota_i = pers.tile([P, P], dt.int32)
        nc.gpsimd.iota(iota_i[:], pattern=[[1, P]], base=0, channel_multiplier=0)
        nc.vector.tensor_copy(out=iota_f[:], in_=iota_i[:])
        make_identity(nc, ident[:])
        nc.vector.tensor_copy(out=ident_bf[:], in_=ident[:])

        cb2v = pers.tile([1, DOUT], f32)
        nc.sync.dma_start(out=cb2v[:], in_=cb2[:])
        nc.gpsimd.partition_broadcast(cb2m[:], cb2v[:])
        if not FL.get("ln_trivial", False):
            nc.sync.dma_start(out=bnt1[:], in_=ln_g[:])
            nc.gpsimd.partition_broadcast(gml[:], bnt1[:])
            nc.sync.dma_start(out=bnt1[:], in_=ln_b[:])
            nc.gpsimd.partition_broadcast(bml[:], bnt1[:])
        if not FL.get("bin_zero", True):
            nc.sync.dma_start(out=bnt1[:], in_=b_in[:])
            nc.gpsimd.partition_broadcast(binm[:], bnt1[:])
        if not FL.get("cb1_zero", True):
            nc.sync.dma_start(out=bnt1[:], in_=cb1[:])
            nc.gpsimd.partition_broadcast(cb1m[:], bnt1[:])

        # zero the uvpair table once (gathers read full 256 B rows)
        with tc.tile_pool(name="uvz", bufs=1) as uvz:
            zt = uvz.tile([P, 32 * UVW], f32)
            nc.gpsimd.memset(zt[:], 0)
            CH = 4096
            for r0 in range(0, NPAIR, CH):
                ch = min(CH, NPAIR - r0)
                nc.sync.dma_start(
                    out=uv[r0:r0 + ch, :].rearrange("(q p) e -> p q e", p=P),
                    in_=zt[:, 0:(ch // P) * UVW].rearrange("p (q e) -> p q e", e=UVW))

        # ================= phase 0: dense + LN + normalize (own nodes) ======
        with tc.tile_pool(name="p0", bufs=2) as p0, \
             tc.tile_pool(name="p0ps", bufs=2, space="PSUM") as p0ps:
            n_batches = GPC // VB + (1 if GPC % VB else 0)
            for mb in range(n_batches):
                v0 = mb * VB
                nv = min(VB, GPC - v0)
                xb = p0.tile([P, VB, 4, P], f32r, tag="xb")
                xTr = xT.rearrange("(k p) (b n) -> p b k n", k=4, p=P, n=P)
                for v in range(nv):
                    nc.sync.dma_start(out=xb[:, v], in_=xTr[:, v0 + v].bitcast(f32r))
                yb = p0.tile([P, VB, D], fp8, tag="yb")
                mu_s = p0.tile([P, VB], f32, tag="mu_s")
                var_s = p0.tile([P, VB], f32, tag="var_s")
                sd_t = p0.tile([P, VB], f32, tag="sd_t")
                istd = p0.tile([P, VB], f32, tag="istd")
                sv_t = p0.tile([P, VB], f32, tag="sv_t")
                nrm_t = p0.tile([P, VB], f32, tag="nrm_t")
                dba = p0.tile([P, VB], f32, tag="dba")
                idv = p0.tile([P, VB], f32, tag="idv")
                sc_t = p0.tile([P, VB], f32, tag="sc_t")
                hsb = []
                hcb = []
                for v in range(nv):
                    ph = p0ps.tile([P, D], f32, tag="ph", space="PSUM")
                    for k in range(4):
                        nc.tensor.matmul(out=ph[:], lhsT=xb[:, v, k, :],
                                         rhs=w_in_sb[:, k, :],
                                         start=(k == 0), stop=(k == 3))
                    h_sb = p0.tile([P, D], f32, tag=f"h{v}")
                    if not FL.get("bin_zero", True):
                        nc.vector.tensor_add(out=h_sb[:], in0=ph[:], in1=binm[:])
                        nc.vector.tensor_scalar_max(out=h_sb[:], in0=h_sb[:], scalar1=0.0)
                    else:
                        nc.vector.tensor_scalar_max(out=h_sb[:], in0=ph[:], scalar1=0.0)
                    nc.vector.reduce_sum(out=mu_s[:, v:v + 1], in_=h_sb[:],
                                         axis=mybir.AxisListType.X)
                    hsb.append(h_sb)
                nc.vector.tensor_scalar_mul(out=mu_s[:, 0:nv], in0=mu_s[:, 0:nv],
                                            scalar1=-1.0 / D)
                for v in range(nv):
                    hc = p0.tile([P, D], f32, tag=f"hc{v}")
                    nc.vector.tensor_scalar_add(out=hc[:], in0=hsb[v][:],
                                                scalar1=mu_s[:, v:v + 1])
                    sq = p0.tile([P, D], f32, tag="sq")
                    nc.scalar.activation(sq[:], hc[:], AF.Square,
                                         accum_out=var_s[:, v:v + 1])
                    hcb.append(hc)
                if FL.get("ln_trivial", True):
                    # d = istd*sqrt(var_s) + eps_n ; scale = istd/d
                    nc.scalar.activation(sd_t[:, 0:nv], var_s[:, 0:nv], AF.Sqrt,
                                         bias=epsln[:], scale=1.0 / D)
                    nc.vector.reciprocal(out=istd[:, 0:nv], in_=sd_t[:, 0:nv])
                    nc.scalar.activation(sv_t[:, 0:nv], var_s[:, 0:nv], AF.Sqrt)
                    nc.vector.tensor_mul(out=nrm_t[:, 0:nv], in0=istd[:, 0:nv],
                                         in1=sv_t[:, 0:nv])
                    nc.vector.tensor_scalar_add(out=dba[:, 0:nv], in0=nrm_t[:, 0:nv],
                                                scalar1=NRM_EPS)
                    nc.vector.reciprocal(out=idv[:, 0:nv], in_=dba[:, 0:nv])
                    nc.vector.tensor_mul(out=sc_t[:, 0:nv], in0=istd[:, 0:nv],
                                         in1=idv[:, 0:nv])
                else:
                    nc.scalar.activation(sd_t[:, 0:nv], var_s[:, 0:nv], AF.Sqrt,
                                         bias=epsln[:], scale=1.0 / D)
                    nc.vector.reciprocal(out=istd[:, 0:nv], in_=sd_t[:, 0:nv])
                    for v in range(nv):
                        hl = p0.tile([P, D], f32, tag=f"hl{v}")
                        nc.vector.scalar_tensor_tensor(
                            out=hl[:], in0=hcb[v][:], scalar=istd[:, v:v + 1],
                            in1=gml[:], op0=OP.mult, op1=OP.mult)
                        nc.vector.tensor_add(out=hl[:], in0=hl[:], in1=bml[:])
                        sq2 = p0.tile([P, D], f32, tag="sq")
                        nc.vector.scalar_tensor_tensor(
                            out=sq2[:], in0=hl[:], scalar=1.0, in1=hl[:],
                            op0=OP.mult, op1=OP.mult,
                            accum_out=nrm_t[:, v:v + 1])
                        hcb[v] = hl
                    nc.scalar.activation(sv_t[:, 0:nv], nrm_t[:, 0:nv], AF.Sqrt)
                    nc.vector.tensor_scalar_add(out=dba[:, 0:nv], in0=sv_t[:, 0:nv],
                                                scalar1=NRM_EPS)
                    nc.vector.reciprocal(out=sc_t[:, 0:nv], in_=dba[:, 0:nv])
                for v in range(nv):
                    m = v0 + v
                    # hn resident (bf16) via Act; Y fp8 via DVE
                    nc.scalar.activation(hn_all[:, m, :], hcb[v][:], AF.Copy,
                                         scale=sc_t[:, v:v + 1])
                    nc.vector.tensor_scalar_mul(out=yb[:, v, :], in0=hcb[v][:],
                                                scalar1=sc_t[:, v:v + 1])
                    nc.vector.tensor_scalar_mul(out=a_own[:, m:m + 1],
                                                in0=hcb[v][:, 0:1],
                                                scalar1=sc_t[:, v:v + 1])
                    nc.vector.tensor_scalar_mul(out=b_own[:, m:m + 1],
                                                in0=hcb[v][:, 1:2],
                                                scalar1=sc_t[:, v:v + 1])
                    nc.vector.tensor_copy(out=d_own[:, m:m + 1],
                                          in_=dba[:, v:v + 1])
                nc.sync.dma_start(
                    out=ccy_in[v0 * P:(v0 + nv) * P, :].rearrange(
                        "(v p) e -> p v e", v=nv, p=P),
                    in_=yb[:, 0:nv, :])

        # ================= AllGather Y =================
        if not skip_cc:
            nc.gpsimd.collective_compute(
                "AllGather", mybir.AluOpType.bypass,
                replica_groups=[list(range(NC))],
                ins=[ccy_in.opt()], outs=[ccy_out.opt()])

        # ---- classifier tail precompute (overlaps the Y AllGather) ----
        # z_pre = (d * hn with head cols zeroed) @ cW1 -- independent of the
        # message-passing layers; phase 5 only adds the rank-2 head update.
        cw1h = pers.tile([2, D], bf16)
        nc.sync.dma_start(out=cw1h[:], in_=cW1[0:2, :])
        zpre_d = dram.tile([NPC, D], bf16, tag="zpre")
        with tc.tile_pool(name="zp", bufs=2) as zp, \
             tc.tile_pool(name="zpps", bufs=2, space="PSUM") as zpps:
            for g in range(GPC):
                htp = zp.tile([P, D], bf16, tag="ht")
                nc.scalar.activation(htp[:], hn_all[:, g, :], AF.Copy,
                                     scale=d_own[:, g:g + 1])
                nc.vector.memset(htp[:, 0:2], 0)
                hTp = zp.tile([P, 4, P], bf16, tag="hT")
                nc.sync.dma_start_transpose(out=hTp[:], in_=htp[:])
                pzp = zpps.tile([P, D], f32, tag="z", space="PSUM")
                for k in range(4):
                    nc.tensor.matmul(out=pzp[:], lhsT=hTp[:, k, :],
                                     rhs=cw1_sb[:, k, :],
                                     start=(k == 0), stop=(k == 3))
                zpb = zp.tile([P, D], bf16, tag="zp")
                nc.vector.tensor_copy(out=zpb[:], in_=pzp[:])
                nc.sync.dma_start(out=zpre_d[g * P:(g + 1) * P, :], in_=zpb[:])

        # ================= phase 3: full aggregation, T =================
        lidx = stack.enter_context(tc.tile_pool(name="lidx", bufs=1))
        uidx_all = lidx.tile([P, GPC, su], i16)
        nc.sync.dma_start(out=uidx_all[:], in_=uidxT.rearrange("g p s -> p g s"))
        dlo = (0, DIAG)                  # diag block range, class 0
        dhi = (B[0], B[0] + DIAG)        # diag block range, class 1
        olo = (DIAG, B[0])               # overflow range, class 0
        ohi = (B[0] + DIAG, BT)          # overflow range, class 1
        with tc.tile_pool(name="p3", bufs=2) as p3, \
             tc.tile_pool(name="p3ps", bufs=2, space="PSUM") as p3ps:
            for g in range(GPC):
                tg = {}
                for y in (0, 1):
                    s = B[y] * P // 16
                    tidx = p3.tile([P, s], i16, tag=f"yi{y}")
                    nc.sync.dma_start(out=tidx[:], in_=yidxT[y][g])
                    t = p3.tile([P, B[y], D], fp8, tag=f"tg{y}")
                    nc.gpsimd.dma_gather(
                        out_ap=t[:], in_ap=Yt[y * HALF:(y + 1) * HALF, :],
                        idxs_ap=tidx[:], num_idxs=B[y] * P,
                        num_idxs_reg=B[y] * P, elem_size=D,
                        single_packet=False)
                    tg[y] = t
                selw = p3.tile([P, BT, P], fp8, tag="selw")
                # diagonal blocks: selw[:, b, :] = omg[:, b] * I
                for (b0, b1) in (dlo, dhi):
                    nc.vector.tensor_tensor(
                        out=selw[:, b0:b1],
                        in0=omg_all[:, g, b0:b1][:, :, None].to_broadcast([P, b1 - b0, P]),
                        in1=ident_bf[:, None, :].to_broadcast([P, b1 - b0, P]),
                        op=OP.mult)
                # overflow blocks: is_equal + weight (on Pool; DVE is the
                # phase-3 bottleneck)
                for ci, (b0, b1) in enumerate((olo, ohi)):
                    o0 = ci * OV[0]
                    nb = b1 - b0
                    nc.vector.tensor_tensor(
                        out=selw[:, b0:b1],
                        in0=srclov_all[:, g, o0:o0 + nb][:, :, None].to_broadcast([P, nb, P]),
                        in1=iota_f[:, None, :].to_broadcast([P, nb, P]),
                        op=OP.is_equal)
                    nc.vector.tensor_tensor(
                        out=selw[:, b0:b1], in0=selw[:, b0:b1],
                        in1=omg_all[:, g, b0:b1][:, :, None].to_broadcast([P, nb, P]),
                        op=OP.mult)
                pm = p3ps.tile([P, D], f32, tag="M", space="PSUM")
                npair = BT // 2
                for pr in range(npair):
                    b0 = 2 * pr
                    y = 0 if b0 < B[0] else 1
                    bl = b0 - (0 if y == 0 else B[0])
                    nc.tensor.matmul(out=pm[:], lhsT=selw[:, b0:b0 + 2, :],
                                     rhs=tg[y][:, bl:bl + 2, :],
                                     start=(pr == 0), stop=(pr == npair - 1),
                                     perf_mode=mybir.MatmulPerfMode.DoubleRow)
                scr = p3.tile([P, D], f32, tag="scr")
                nc.vector.tensor_tensor(out=scr[:], in0=pm[:],
                                        in1=hn_all[:, g, :], op=OP.mult)
                nc.vector.reduce_sum(out=ang1[:, g:g + 1], in_=scr[:],
                                     axis=mybir.AxisListType.X)
                hsum = p3.tile([P, 1], f32, tag="hsum")
                nc.vector.reduce_sum(out=hsum[:], in_=scr[:, 0:2],
                                     axis=mybir.AxisListType.X)
                nc.vector.tensor_sub(out=T_own[:, g:g + 1], in0=ang1[:, g:g + 1],
                                     in1=hsum[:])

        # ================= layers =================
        for layer in (1, 2, 3):
            if layer == 1:
                ang_src = ang1
            else:
                GB = 2
                with tc.tile_pool(name=f"l{layer}", bufs=2) as lp, \
                     tc.tile_pool(name=f"l{layer}ps", bufs=2, space="PSUM") as lps:
                  for g0 in range(0, GPC, GB):
                    gn = min(GB, GPC - g0)
                    tuvb = lp.tile([P, GB * BT, UVW], f32, tag="tuv")
                    nc.gpsimd.dma_gather(
                        out_ap=tuvb[:, 0:gn * BT, :], in_ap=uv[:],
                        idxs_ap=uidx_all[:, g0:g0 + gn, :].rearrange(
                            "p g s -> p (g s)"),
                        num_idxs=gn * BT * P,
                        num_idxs_reg=gn * BT * P, elem_size=UVW,
                        single_packet=False)
                    for gi in range(gn):
                        g = g0 + gi
                        # parity select: uvc[:, :, c] = tuv[..c]*pinv + tuv[..2+c]*podd
                        uvc = lp.tile([P, BT, 2], bf16, tag="uvc")
                        uvo = lp.tile([P, BT, 2], bf16, tag="uvo")
                        for c in (0, 1):
                            nc.vector.tensor_tensor(
                                out=uvc[:, :, c],
                                in0=tuvb[:, gi * BT:(gi + 1) * BT, c],
                                in1=pinv_all[:, g, :], op=OP.mult)
                            nc.vector.tensor_tensor(
                                out=uvo[:, :, c],
                                in0=tuvb[:, gi * BT:(gi + 1) * BT, 2 + c],
                                in1=podd_all[:, g, :], op=OP.mult)
                        nc.vector.tensor_add(out=uvc[:], in0=uvc[:], in1=uvo[:])
                        # diagonal reduce on DVE
                        wm = lp.tile([P, BT], f32, tag="wm")
                        pa = lp.tile([P, 4], f32, tag="pa")
                        for c, dest in ((0, P_all), (1, Q_all)):
                            nc.vector.tensor_tensor(
                                out=wm[:], in0=uvc[:, :, c],
                                in1=omg_all[:, g, :], op=OP.mult)
                            nc.vector.reduce_sum(out=pa[:, 0:1],
                                                 in_=wm[:, dlo[0]:dlo[1]],
                                                 axis=mybir.AxisListType.X)
                            nc.vector.reduce_sum(out=pa[:, 1:2],
                                                 in_=wm[:, dhi[0]:dhi[1]],
                                                 axis=mybir.AxisListType.X)
                            nc.vector.tensor_add(out=dest[:, g:g + 1],
                                                 in0=pa[:, 0:1], in1=pa[:, 1:2])
                        # overflow blocks on PE
                        sow = lp.tile([P, OVT, P], bf16, tag="sow")
                        nc.vector.tensor_tensor(
                            out=sow[:],
                            in0=srclov_all[:, g, :][:, :, None].to_broadcast([P, OVT, P]),
                            in1=iota_f[:, None, :].to_broadcast([P, OVT, P]),
                            op=OP.is_equal)
                        omg_ov = lp.tile([P, OVT], f32, tag="omg_ov")
                        nc.vector.tensor_copy(out=omg_ov[:, 0:OV[0]],
                                              in_=omg_all[:, g, olo[0]:olo[1]])
                        nc.vector.tensor_copy(out=omg_ov[:, OV[0]:OVT],
                                              in_=omg_all[:, g, ohi[0]:ohi[1]])
                        nc.vector.tensor_tensor(
                            out=sow[:], in0=sow[:],
                            in1=omg_ov[:, :, None].to_broadcast([P, OVT, P]),
                            op=OP.mult)
                        pq = lps.tile([P, 2], f32, tag="PQ", space="PSUM")
                        for j, (b0, b1) in enumerate((olo, ohi)):
                            for bb in range(b1 - b0):
                                oj = j * OV[0] + bb
                                nc.tensor.matmul(
                                    out=pq[:], lhsT=sow[:, oj, :],
                                    rhs=uvc[:, b0 + bb, :],
                                    start=(oj == 0), stop=(oj == OVT - 1))
                        nc.vector.tensor_add(out=P_all[:, g:g + 1],
                                             in0=P_all[:, g:g + 1], in1=pq[:, 0:1])
                        nc.vector.tensor_add(out=Q_all[:, g:g + 1],
                                             in0=Q_all[:, g:g + 1], in1=pq[:, 1:2])
                nc.vector.tensor_mul(out=r1[:], in0=P_all[:], in1=a_own[:])
                nc.vector.tensor_mul(out=r2[:], in0=Q_all[:], in1=b_own[:])
                nc.vector.tensor_add(out=r1[:], in0=r1[:], in1=r2[:])
                nc.vector.tensor_add(out=angL[:], in0=T_own[:], in1=r1[:])
                ang_src = angL
            nc.scalar.activation(c_t[:], ang_src[:], AF.Sin, bias=halfpi[:])
            nc.scalar.activation(s_t[:], ang_src[:], AF.Sin)
            nc.vector.tensor_mul(out=r1[:], in0=c_t[:], in1=a_own[:])
            nc.vector.tensor_mul(out=r2[:], in0=s_t[:], in1=b_own[:])
            nc.vector.tensor_mul(out=r3[:], in0=s_t[:], in1=a_own[:])
            nc.vector.tensor_mul(out=r4[:], in0=c_t[:], in1=b_own[:])
            nc.vector.tensor_sub(out=a_own[:], in0=r1[:], in1=r2[:])
            nc.vector.tensor_add(out=b_own[:], in0=r3[:], in1=r4[:])
            if layer < 3:
                nc.vector.tensor_copy(out=uvp[:, :, 0:1], in_=a_own[:, :, None])
                nc.vector.tensor_copy(out=uvp[:, :, 1:2], in_=b_own[:, :, None])
                nc.gpsimd.dma_start(out=ccab_in[:], in_=uvp[:].rearrange("p g e -> p (g e)"))
                if not skip_cc:
                    nc.gpsimd.collective_compute(
                        "AllGather", mybir.AluOpType.bypass,
                        replica_groups=[list(range(NC))],
                        ins=[ccab_in.opt()], outs=[ccab_out.opt()])
                # scatter into pair table: node n=(r*NPC + g*128 + p),
                # row n>>1 = r*NPC/2 + g*64 + (p>>1), col (p&1)*2 + e
                HNPC = NPC // 2
                for rr in range(NC):
                    ccr = ccab_out[rr].rearrange(
                        "(q par) (g e) -> par q g e", q=64, par=2, g=GPC, e=2)
                    for par in (0, 1):
                        nc.sync.dma_start(
                            out=uv[rr * HNPC:(rr + 1) * HNPC,
                                   par * 2:par * 2 + 2].rearrange(
                                "(g q) e -> q g e", g=GPC, q=64),
                            in_=ccr[par])

        # ================= phase 5: classifier =================
        # pass A: per group -> logits (PE/DVE + Act stays on Copy; relu on DVE)
        logits = pers.tile([P, GPC, DOUT], f32)
        with tc.tile_pool(name="p5", bufs=4) as p5, \
             tc.tile_pool(name="p5ps", bufs=2, space="PSUM") as p5ps:
            for g in range(GPC):
                abd = p5.tile([P, 2], bf16, tag="abd")
                nc.vector.tensor_mul(out=abd[:, 0:1], in0=a_own[:, g:g + 1],
                                     in1=d_own[:, g:g + 1])
                nc.vector.tensor_mul(out=abd[:, 1:2], in0=b_own[:, g:g + 1],
                                     in1=d_own[:, g:g + 1])
                ptr2 = p5ps.tile([2, P], bf16, tag="abT", space="PSUM")
                nc.tensor.transpose(out=ptr2[:], in_=abd[:], identity=ident_bf[:])
                abT = p5.tile([2, P], bf16, tag="abTs")
                nc.vector.tensor_copy(out=abT[:], in_=ptr2[:])
                pz = p5ps.tile([P, D], f32, tag="z", space="PSUM")
                nc.tensor.matmul(out=pz[:], lhsT=abT[:], rhs=cw1h[:],
                                 start=True, stop=True)
                zl = p5.tile([P, D], bf16, tag="zl")
                nc.sync.dma_start(out=zl[:], in_=zpre_d[g * P:(g + 1) * P, :])
                z_sb = p5.tile([P, D], bf16, tag="z_sb")
                nc.vector.tensor_add(out=z_sb[:], in0=pz[:], in1=zl[:])
                nc.vector.tensor_scalar_max(out=z_sb[:], in0=z_sb[:],
                                            scalar1=0.0)
                zT = p5.tile([P, 4, P], bf16, tag="zT")
                nc.sync.dma_start_transpose(out=zT[:], in_=z_sb[:])
                plg = p5ps.tile([P, DOUT], f32, tag="lg", space="PSUM")
                for k in range(4):
                    nc.tensor.matmul(out=plg[:], lhsT=zT[:, k, :],
                                     rhs=cw2_sb[:, k, :],
                                     start=(k == 0), stop=(k == 3))
                nc.vector.tensor_add(out=logits[:, g, :], in0=plg[:], in1=cb2m[:])
        # pass B: log_softmax with activation functions batched per pass
        with tc.tile_pool(name="p5b", bufs=1) as p5b:
            sh = p5b.tile([P, GPC, DOUT], f32)
            ex = p5b.tile([P, GPC, DOUT], f32)
            mx = p5b.tile([P, GPC], f32)
            se = p5b.tile([P, GPC], f32)
            ls = p5b.tile([P, GPC], f32)
            for g in range(GPC):
                nc.vector.reduce_max(out=mx[:, g:g + 1], in_=logits[:, g, :],
                                     axis=mybir.AxisListType.X)
                nc.vector.tensor_scalar_sub(out=sh[:, g, :], in0=logits[:, g, :],
                                            scalar1=mx[:, g:g + 1])
            for g in range(GPC):
                nc.scalar.activation(ex[:, g, :], sh[:, g, :], AF.Exp,
                                     accum_out=se[:, g:g + 1])
            nc.scalar.activation(ls[:], se[:], AF.Ln)
            for g in range(GPC):
                ob = p5b.tile([P, DOUT], f32, tag=f"ob{g % 2}")
                nc.vector.tensor_scalar_sub(out=ob[:], in0=sh[:, g, :],
                                            scalar1=ls[:, g:g + 1])
                nc.sync.dma_start(out=out[g * P:(g + 1) * P, :], in_=ob[:])

    nc.compile()
    return nc


# ---------------------------------------------------------------- in_maps

def _fold_bn(weights):
    al = weights["bn_g"] / np.sqrt(weights["bn_var"] + BN_EPS)
    be = weights["bn_b"] - weights["bn_mean"] * al
    cW2p = (weights["cW2"] * al[:, None]).astype(np.float32)
    cb2p = (be @ weights["cW2"] + weights["cb2"]).astype(np.float32)
    return cW2p, cb2p


def make_in_maps(cfg, percore, weights):
    cW2p, cb2p = _fold_bn(weights)
    ins = []
    for r in range(cfg.NC):
        pc = percore[r]
        m = dict(
            xT=pc["xT"],
            W_in=weights["W_in"], b_in=weights["b_in"][None, :],
            ln_g=weights["ln_g"][None, :], ln_b=weights["ln_b"][None, :],
            cW1=weights["cW1"].astype(ml_dtypes.bfloat16),
            cb1=weights["cb1"][None, :],
            cW2=cW2p.astype(ml_dtypes.bfloat16),
            cb2=cb2p[None, :],
            omg=pc["omg"], podd=pc["podd"], srcl_ov=pc["srcl_ov"],
            yidx0=pc["yidx0"], yidx1=pc["yidx1"], uidx=pc["uidx"],
        )
        ins.append(m)
    return ins


def assemble_output(cfg, results, n):
    chunks = [results[r]["out"] for r in range(cfg.NC)]
    full = np.concatenate(chunks, axis=0)
    return full[:n]


def _make_cfg_flags(w):
    return dict(
        bin_zero=bool(np.all(w["b_in"] == 0)),
        ln_trivial=bool(np.all(w["ln_g"] == 1) and np.all(w["ln_b"] == 0)),
        cb1_zero=bool(np.all(w["cb1"] == 0)),
    )


# ---------------------------------------------------------------- entry point

def kernel(**inputs):
    """Full-input GNN forward on 8 TRN2 NeuronCores; returns [N, 40] fp32."""
    x = np.asarray(inputs["x"], np.float32)
    edge_src = np.asarray(inputs["edge_src"])
    edge_dst = np.asarray(inputs["edge_dst"])
    w = {k: np.asarray(inputs[k], np.float32) for k in
         ["W_in", "b_in", "ln_g", "ln_b", "cW1", "cb1", "bn_g", "bn_b",
          "bn_mean", "bn_var", "cW2", "cb2"]}
    N = x.shape[0]

    cfg, percore = host_prep(x, edge_src, edge_dst, n_cores=8)
    cfg.flags = _make_cfg_flags(w)
    nc = build_nc(cfg)
    in_maps = make_in_maps(cfg, percore, w)

    from concourse.bass_utils import run_bass_kernel_spmd
    res = run_bass_kernel_spmd(nc, in_maps, core_ids=list(range(cfg.NC)))
    return assemble_output(cfg, res.results, N).astype(np.float32)


def estimate_exec_ns(inputs):
    """Tile cost-model (TimelineSim) estimate of the per-core program span.

    Unlike v1, the collectives (one 25.7 MB fp8 AllGather of Y plus two
    50 KB AllGathers of the rotated head values) ARE included via the cost
    model's collective bandwidth model."""
    x = np.asarray(inputs["x"], np.float32)
    w = {k: np.asarray(inputs[k], np.float32) for k in
         ["W_in", "b_in", "ln_g", "ln_b", "cW1", "cb1", "bn_g", "bn_b",
          "bn_mean", "bn_var", "cW2", "cb2"]}
    cfg, _ = host_prep(x, np.asarray(inputs["edge_src"]),
                       np.asarray(inputs["edge_dst"]), n_cores=8)
    cfg.flags = _make_cfg_flags(w)
    nc2 = build_nc(cfg, skip_cc=False)
    from concourse.timeline_sim import TimelineSim
    tl = TimelineSim(nc2, trace=False)
    ns = tl.simulate()
    return int(ns)
